# revision 17
# baseline (speedup 1.0000x reference)
"""Trainium2 Bass kernel for PVT-style MHSA with spatial reduction.

Problem (hardcoded): B=4, C=384, H=W=64, NH=8 heads, HD=48, SR=2.
  q = Wq@x;  xsr = conv2x2s2(x, Wsr)+bsr;  k = (Wk@xsr + pos)*scale;  v = Wv@xsr
  attn = softmax(q^T k);  out = Wp@(v attn) + bp

Sharding: 8 cores = (batch b, query-half s).  Each core computes the full
conv/k/v for its batch (duplicated across the 2 cores of a batch) and
attention + projection for its 2048 queries.  No collectives.

v2 changes vs v1:
  - exp split across engines: most key-tiles take ACT exp; `n_dve` tiles per
    block take a Schraudolph bit-trick exp on the DVE (one fused
    tensor_scalar: int16(x*A+B) whose bits are read back as bf16).
  - reciprocal_approx_fast instead of the iterative DVE reciprocal
    (3354ns -> ~660ns per rowsum row).
  - bf16 activations on the q/conv inputs + all attention-side tensors
    (vt/e/o/proj); k-chain (wk/wv/xsr) kept fp32r for logit accuracy.
  - bf16 output, DMA'd per query block on alternating rings.

Device notes (kept from v1):
  - heads padded 48 -> 64 channels; head-pair hp occupies one 128-row tile.
  - attention computed transposed: attnT[m, n]; key-axis softmax reduction
    rides the AV matmul via an all-ones column in v^T at head-local col 32;
    QK^T pairs row-packed, AV pairs col-packed via tile_position.
  - rowsum reciprocal DRAM-bounce partition-broadcast, one tensor_mul.
"""

import threading

import numpy as np
import ml_dtypes

import concourse.bass as bass
import concourse.mybir as mybir
import concourse.tile as tile
from concourse import bacc
from concourse.bass import ts
from concourse.bass_utils import run_bass_kernel_spmd

B, C, H, W = 4, 384, 64, 64
NH, HD, SR = 8, 48, 2
SCALE = HD ** -0.5
Hs, Ws = H // SR, W // SR
NK = Hs * Ws            # 1024 keys
N = H * W               # 4096 queries / batch
NQ = N // 2             # 2048 queries / core
CT = C // 128           # 3 c-tiles
HP = NH // 2            # 4 head-pair tiles
NB = NQ // 512          # 4 query blocks / core
MT = NK // 128          # 8 key tiles

F32 = mybir.dt.float32
F32R = mybir.dt.float32r
BF16 = mybir.dt.bfloat16
I16 = mybir.dt.int16
AF = mybir.ActivationFunctionType
ALU = mybir.AluOpType

# Schraudolph exp in bf16-bit space: exp(x) ~ bits_bf16(int16(x*A + B))
EXP_A = 128.0 / float(np.log(2.0))
EXP_B = 127.0 * 128.0 - 128.0 * 0.04368

DEFAULT_CFG = dict(
    psa_bufs=4, qk_bufs=2, av_bufs=2, pr_bufs=2, e_bufs=2, r_bufs=3, dr_bufs=3,
    n_dve=3, debug=False,
)
# which key-tiles (mi) run the DVE bit-trick exp, by n_dve
DVE_MI = {0: (), 1: (3,), 2: (2, 5), 3: (1, 4, 6), 4: (1, 3, 5, 7)}


def build_program(**cfg):
    cfg = {**DEFAULT_CFG, **cfg}
    dve_mi = DVE_MI[cfg["n_dve"]]
    nc = bacc.Bacc(None, target_bir_lowering=False)

    xf = nc.dram_tensor("xf", [128, 2, CT, 4, 512], BF16, kind="ExternalInput")
    xq = nc.dram_tensor("xq", [128, NB, CT, 512], BF16, kind="ExternalInput")
    wq = nc.dram_tensor("wq", [128, CT, 512], BF16, kind="ExternalInput")
    wk = nc.dram_tensor("wk", [128, CT, 512], F32R, kind="ExternalInput")
    wv = nc.dram_tensor("wv", [128, CT, 512], F32R, kind="ExternalInput")
    wsr = nc.dram_tensor("wsr", [128, 12, C], BF16, kind="ExternalInput")
    wp = nc.dram_tensor("wp", [128, 4, C], BF16, kind="ExternalInput")
    pos = nc.dram_tensor("pos", [128, HP, NK], F32, kind="ExternalInput")
    bsr = nc.dram_tensor("bsr", [128, CT], F32, kind="ExternalInput")
    bp = nc.dram_tensor("bp", [128, CT], F32, kind="ExternalInput")
    out = nc.dram_tensor("out", [128, CT, NB, 512], BF16, kind="ExternalOutput")
    if cfg["debug"]:
        xsr_d = nc.dram_tensor("xsr_d", [128, CT, NK], F32R, kind="ExternalOutput")
        q_d = nc.dram_tensor("q_d", [128, HP, NQ], F32R, kind="ExternalOutput")
        k_d = nc.dram_tensor("k_d", [128, HP, NK], F32R, kind="ExternalOutput")
        vt_d = nc.dram_tensor("vt_d", [128, MT, 512], BF16, kind="ExternalOutput")
        e_d = nc.dram_tensor("e_d", [128, MT, 1024], BF16, kind="ExternalOutput")
        rb_d = nc.dram_tensor("rb_d", [128, 512], F32, kind="ExternalOutput")
        o_d = nc.dram_tensor("o_d", [128, HP, NQ], BF16, kind="ExternalOutput")

    with tile.TileContext(nc) as tc:
        with (
            tc.tile_pool(name="constp", bufs=1) as constp,
            tc.tile_pool(name="actp", bufs=1) as actp,
        ):
            wk_sb = constp.tile([128, CT, 512], F32R, name="wk_sb")
            wv_sb = constp.tile([128, CT, 512], F32R, name="wv_sb")
            wp_sb = constp.tile([128, 4, C], BF16, name="wp_sb")
            bsr_sb = constp.tile([128, CT], F32, name="bsr_sb")
            bp_sb = constp.tile([128, CT], F32, name="bp_sb")

            q_sb = actp.tile([128, HP, NQ], F32R, name="q_sb")
            k_sb = actp.tile([128, HP, NK], F32R, name="k_sb")
            vt_sb = actp.tile([128, MT, 512], BF16, name="vt_sb")

            # ---- phase A: conv + projections -----------------------------
            with (
                tc.tile_pool(name="aload", bufs=1) as aload,
                tc.tile_pool(name="psA", bufs=cfg["psa_bufs"], space="PSUM") as psA,
            ):
                xf_sb = aload.tile([128, 2, CT, 4, 512], BF16, name="xf_sb")
                xq_sb = aload.tile([128, NB, CT, 512], BF16, name="xq_sb")
                wq_sb = aload.tile([128, CT, 512], BF16, name="wq_sb")
                wsr_sb = aload.tile([128, 12, C], BF16, name="wsr_sb")
                pos_sb = aload.tile([128, HP, NK], F32, name="pos_sb")
                xsr_sb = aload.tile([128, CT, NK], F32R, name="xsr_sb")

                # ACT HWDGE ring: weights/bias/pos (ACT is idle until exps)
                nc.scalar.dma_start(wsr_sb[:], wsr[:])
                nc.scalar.dma_start(wk_sb[:], wk[:])
                nc.scalar.dma_start(wq_sb[:], wq[:])
                nc.scalar.dma_start(bsr_sb[:], bsr[:])
                nc.scalar.dma_start(pos_sb[:], pos[:])
                nc.scalar.dma_start(wv_sb[:], wv[:])
                nc.scalar.dma_start(wp_sb[:], wp[:])
                nc.scalar.dma_start(bp_sb[:], bp[:])
                # SP HWDGE ring: activations, ordered by first use
                nc.sync.dma_start(xf_sb[:, 0], xf[:, 0])
                nc.sync.dma_start(xq_sb[:, 0], xq[:, 0])
                nc.sync.dma_start(xf_sb[:, 1], xf[:, 1])
                for nb in range(1, NB):
                    nc.sync.dma_start(xq_sb[:, nb], xq[:, nb])

                def emit_conv(mb):
                    for ot in range(CT):
                        p = psA.tile([128, 512], F32, name="pa", tag="pa")
                        n_mm = 0
                        for didj in range(4):
                            for ci in range(CT):
                                nc.tensor.matmul(
                                    p[:],
                                    wsr_sb[:, didj * CT + ci, ts(ot, 128)],
                                    xf_sb[:, mb, ci, didj, :],
                                    start=(n_mm == 0),
                                    stop=(n_mm == 11),
                                )
                                n_mm += 1
                        nc.vector.tensor_scalar_add(
                            xsr_sb[:, ot, ts(mb, 512)], p[:], bsr_sb[:, ot : ot + 1]
                        )

                def emit_k(hp, mb):
                    p = psA.tile([128, 512], F32, name="pa", tag="pa")
                    for ci in range(CT):
                        nc.tensor.matmul(
                            p[:],
                            wk_sb[:, ci, ts(hp, 128)],
                            xsr_sb[:, ci, ts(mb, 512)],
                            start=(ci == 0),
                            stop=(ci == CT - 1),
                        )
                    nc.vector.tensor_add(
                        k_sb[:, hp, ts(mb, 512)], p[:], pos_sb[:, hp, ts(mb, 512)]
                    )

                def emit_q(ot, nb):
                    p = psA.tile([128, 512], F32, name="pa", tag="pa")
                    for ci in range(CT):
                        nc.tensor.matmul(
                            p[:],
                            wq_sb[:, ci, ts(ot, 128)],
                            xq_sb[:, nb, ci, :],
                            start=(ci == 0),
                            stop=(ci == CT - 1),
                        )
                    nc.any.tensor_copy(q_sb[:, ot, ts(nb, 512)], p[:])

                def emit_vt(mi):
                    p = psA.tile([128, 512], F32, name="pa", tag="pa")
                    for ci in range(CT):
                        nc.tensor.matmul(
                            p[:],
                            xsr_sb[:, ci, ts(mi, 128)],
                            wv_sb[:, ci, :],
                            start=(ci == 0),
                            stop=(ci == CT - 1),
                        )
                    nc.any.tensor_copy(vt_sb[:, mi, :], p[:])
                    base = vt_sb[:]
                    ones_ap = bass.AP(
                        tensor=base.tensor,
                        offset=base.offset + mi * 512 + 32,
                        ap=[base.ap[0], [64, NH]],
                    )
                    nc.gpsimd.memset(ones_ap, 1.0)

                # emission order shapes the schedule: unblock (hp0, nb0)
                # attention as early as possible
                emit_conv(0)
                for hp in range(HP):
                    emit_k(hp, 0)
                emit_q(0, 0)
                emit_conv(1)
                for hp in range(HP):
                    emit_k(hp, 1)
                for mi in range(4):
                    emit_vt(mi)
                for ot in range(1, HP):
                    emit_q(ot, 0)
                for mi in range(4, MT):
                    emit_vt(mi)
                for nb in range(1, NB):
                    for ot in range(HP):
                        emit_q(ot, nb)
                if cfg["debug"]:
                    nc.sync.dma_start(xsr_d[:], xsr_sb[:])

            # ---- phase B: attention + projection -------------------------
            # software-pipelined: step s runs QK+exp(block s) interleaved
            # per key-tile with AV(block s-1); rowsum recip+bounce for block
            # s-1; normalize-mul for block s-2 (hides the broadcast DMA).
            with (
                tc.tile_pool(name="bpool", bufs=1) as bpool,
                tc.tile_pool(name="epool", bufs=cfg["e_bufs"]) as epool,
                tc.tile_pool(name="rpool", bufs=cfg["r_bufs"]) as rpool,
                tc.tile_pool(name="opool", bufs=2) as opool,
                tc.tile_pool(name="drp", bufs=cfg["dr_bufs"], space="DRAM") as drp,
                tc.tile_pool(name="qkps", bufs=cfg["qk_bufs"], space="PSUM") as qkps,
                tc.tile_pool(name="avps", bufs=cfg["av_bufs"], space="PSUM") as avps,
                tc.tile_pool(name="prps", bufs=cfg["pr_bufs"], space="PSUM") as prps,
            ):
                o_sb = bpool.tile([128, HP, NQ], BF16, name="o_sb")
                blocks = [(nb, hp) for nb in range(NB) for hp in range(HP)]
                NBLK = len(blocks)
                e_tiles = [None] * NBLK
                oav_tiles = [None] * NBLK
                rb_tiles = [None] * NBLK

                def emit_qk_exp(b, mi):
                    nb, hp = blocks[b]
                    e_sb = e_tiles[b]
                    qk = qkps.tile([128, 1024], F32, name="qk", tag="qk")
                    nc.tensor.matmul(
                        qk[:, 0:512],
                        k_sb[0:64, hp, ts(mi, 128)],
                        q_sb[0:64, hp, ts(nb, 512)],
                        start=True,
                        stop=True,
                        tile_position=(0, 0),
                    )
                    nc.tensor.matmul(
                        qk[:, 512:1024],
                        k_sb[64:128, hp, ts(mi, 128)],
                        q_sb[64:128, hp, ts(nb, 512)],
                        start=True,
                        stop=True,
                        tile_position=(64, 0),
                    )
                    if mi in dve_mi:
                        nc.vector.tensor_scalar(
                            e_sb[:, mi, :].bitcast(I16),
                            qk[:],
                            EXP_A,
                            EXP_B,
                            ALU.mult,
                            ALU.add,
                        )
                    else:
                        nc.scalar.activation(
                            out=e_sb[:, mi, :], in_=qk[:], func=AF.Exp
                        )

                def emit_av(b, mi):
                    nb, hp = blocks[b]
                    e_sb = e_tiles[b]
                    oav = oav_tiles[b]
                    nc.tensor.matmul(
                        oav[0:64, :],
                        vt_sb[:, mi, 128 * hp : 128 * hp + 64],
                        e_sb[:, mi, 0:512],
                        start=(mi == 0),
                        stop=(mi == MT - 1),
                        tile_position=(0, 0),
                        skip_group_check=True,
                    )
                    nc.tensor.matmul(
                        oav[64:128, :],
                        vt_sb[:, mi, 128 * hp + 64 : 128 * (hp + 1)],
                        e_sb[:, mi, 512:1024],
                        start=(mi == 0),
                        stop=(mi == MT - 1),
                        tile_position=(0, 64),
                        skip_group_check=True,
                    )

                def emit_rowsum(b):
                    # copy the two rowsum rows out of PSUM, DRAM-bounce them
                    # into a 128-partition broadcast; reciprocal happens
                    # full-partition-aligned later (custom-DVE ops silently
                    # corrupt on HW with partition-offset APs).
                    oav = oav_tiles[b]
                    r2 = rpool.tile([33, 512], F32, name="r2", tag="r2")
                    nc.vector.tensor_copy(r2[0:1, :], oav[32:33, :])
                    nc.vector.tensor_copy(r2[32:33, :], oav[96:97, :])
                    r2d = drp.tile([2, 512], F32, name="r2d", tag="r2d")
                    nc.gpsimd.dma_start(r2d[0:1], r2[0:1, :])
                    nc.gpsimd.dma_start(r2d[1:2], r2[32:33, :])
                    rb = rpool.tile([128, 512], F32, name="rb", tag="rb")
                    nc.gpsimd.dma_start(
                        rb[0:64, :].unsqueeze(1),
                        r2d[0:1, :].partition_broadcast(64),
                    )
                    nc.gpsimd.dma_start(
                        rb[64:128, :].unsqueeze(1),
                        r2d[1:2, :].partition_broadcast(64),
                    )
                    rb_tiles[b] = rb

                def emit_mul_proj(b):
                    nb, hp = blocks[b]
                    rbi = rpool.tile([128, 512], F32, name="rbi", tag="rbi")
                    nc.vector.reciprocal_approx_fast(
                        out=rbi[:], in_=rb_tiles[b][:]
                    )
                    nc.vector.tensor_mul(
                        o_sb[:, hp, ts(nb, 512)], oav_tiles[b][:], rbi[:]
                    )
                    if hp == HP - 1:
                        outp = opool.tile(
                            [128, CT, 512], BF16, name="outp", tag="outp"
                        )
                        for ot in range(CT):
                            p = prps.tile([128, 512], F32, name="pp", tag="pp")
                            for d in range(4):
                                nc.tensor.matmul(
                                    p[:],
                                    wp_sb[:, d, ts(ot, 128)],
                                    o_sb[:, d, ts(nb, 512)],
                                    start=(d == 0),
                                    stop=(d == 3),
                                )
                            nc.any.tensor_scalar_add(
                                outp[:, ot, :], p[:], bp_sb[:, ot : ot + 1]
                            )
                        ring = nc.sync if nb % 2 == 0 else nc.scalar
                        ring.dma_start(out[:, :, nb], outp[:])

                if cfg["debug"]:
                    nc.sync.dma_start(q_d[:], q_sb[:])
                    nc.sync.dma_start(k_d[:], k_sb[:])
                    nc.sync.dma_start(vt_d[:], vt_sb[:])

                for s in range(NBLK + 2):
                    if s < NBLK:
                        e_tiles[s] = epool.tile(
                            [128, MT, 1024], BF16, name="e", tag="e"
                        )
                    if 0 <= s - 1 < NBLK:
                        oav_tiles[s - 1] = avps.tile(
                            [128, 512], F32, name="oav", tag="oav"
                        )
                    for mi in range(MT):
                        if s < NBLK:
                            emit_qk_exp(s, mi)
                        if s - 1 >= 0 and s - 1 < NBLK:
                            emit_av(s - 1, mi)
                    if 0 <= s - 1 < NBLK:
                        emit_rowsum(s - 1)
                    if 0 <= s - 2 < NBLK:
                        emit_mul_proj(s - 2)
                    if cfg["debug"] and s == 1:
                        nc.sync.dma_start(e_d[:], e_tiles[0][:])
                        nc.sync.dma_start(rb_d[:], rb_tiles[0][:])
                if cfg["debug"]:
                    nc.sync.dma_start(o_d[:], o_sb[:])

    nc.compile()
    return nc


def _pad_cols(w):
    """[C, C] weight -> [C, 512]: col 64h+j = w[48h+j, :] (j < 48)."""
    wt = np.zeros((C, NH * 64), np.float32)
    for h in range(NH):
        wt[:, 64 * h : 64 * h + HD] = w[HD * h : HD * (h + 1), :].T
    return wt


# v/proj head-local channel placement: the ones column sits at local col 32 so
# the rowsum lands on a 32-aligned PSUM partition; channel d -> col d (d<32)
# else d+1
_VCOL = np.array([d if d < 32 else d + 1 for d in range(HD)])


def _ctile(w):
    """[C, F] -> [128, CT, F] (partition-major c-tiles)."""
    return np.ascontiguousarray(w.reshape(CT, 128, -1).transpose(1, 0, 2))


def _bf16(a):
    return np.ascontiguousarray(a).astype(ml_dtypes.bfloat16)


def prep_inputs(inputs):
    x = np.ascontiguousarray(np.asarray(inputs["x"], np.float32))
    Wq = np.asarray(inputs["Wq"], np.float32)
    Wk = np.asarray(inputs["Wk"], np.float32)
    Wv = np.asarray(inputs["Wv"], np.float32)
    Wsr = np.asarray(inputs["Wsr"], np.float32)
    bsr = np.asarray(inputs["bsr"], np.float32)
    Wp = np.asarray(inputs["Wp"], np.float32)
    bp = np.asarray(inputs["bp"], np.float32)
    rel_h = np.asarray(inputs["rel_h"], np.float32)
    rel_w = np.asarray(inputs["rel_w"], np.float32)

    wq_t = _bf16(_ctile(_pad_cols(Wq)))
    wk_t = _ctile(_pad_cols(Wk) * SCALE)
    wv_pad = np.zeros((C, NH * 64), np.float32)
    for h in range(NH):
        wv_pad[:, 64 * h + _VCOL] = Wv[HD * h : HD * (h + 1), :].T
    wv_t = _ctile(wv_pad)
    # conv weights: rows ordered (di, dj, c) -> [128, 12, C] (didj, ci) tiles
    wsr_t = _bf16(
        Wsr.transpose(2, 3, 1, 0).reshape(12, 128, C).transpose(1, 0, 2)
    )
    # proj weights: row 64h + vcol(j) = Wp[:, 48h+j] -> [128, 4, C]
    wp_t = np.zeros((NH * 64, C), np.float32)
    for h in range(NH):
        wp_t[64 * h + _VCOL, :] = Wp[:, HD * h : HD * (h + 1)].T
    wp_t = _bf16(wp_t.reshape(4, 128, C).transpose(1, 0, 2))
    # positional bias, pre-scaled, padded to 64-channel heads -> [128, HP, NK]
    pos_flat = (rel_h + rel_w).reshape(NH, HD, NK).astype(np.float32) * SCALE
    pos_t = np.zeros((NH * 64, NK), np.float32)
    for h in range(NH):
        pos_t[64 * h : 64 * h + HD, :] = pos_flat[h]
    pos_t = np.ascontiguousarray(pos_t.reshape(HP, 128, NK).transpose(1, 0, 2))
    bsr_t = np.ascontiguousarray(bsr.reshape(CT, 128).T)
    bp_t = np.ascontiguousarray(bp.reshape(CT, 128).T)

    in_maps = []
    for core in range(8):
        b, s = core // 2, core % 2
        xb = x[b].reshape(C, N)
        # conv planes, deinterleaved: [p, mb, ci, didj, ph_local*32+pw] =
        # x[ci*128+p, 2*(mb*16+ph_local)+di, 2*pw+dj]
        xf_t = _bf16(
            x[b]
            .reshape(C, Hs, SR, Ws, SR)          # c, ph, di, pw, dj
            .transpose(0, 2, 4, 1, 3)            # c, di, dj, ph, pw
            .reshape(C, 4, 2, Hs // 2, Ws)       # c, didj, mb, ph_l, pw
            .transpose(0, 2, 1, 3, 4)            # c, mb, didj, ph_l, pw
            .reshape(CT, 128, 2, 4, 512)
            .transpose(1, 2, 0, 3, 4)
        )
        xq_t = _bf16(
            xb[:, s * NQ : (s + 1) * NQ]
            .reshape(CT, 128, NB, 512)
            .transpose(1, 2, 0, 3)
        )
        in_maps.append(
            {
                "xf": xf_t,
                "xq": xq_t,
                "wq": wq_t,
                "wk": wk_t,
                "wv": wv_t,
                "wsr": wsr_t,
                "wp": wp_t,
                "pos": pos_t,
                "bsr": bsr_t,
                "bp": bp_t,
            }
        )
    return in_maps


def assemble_output(results):
    out = np.empty((B, C, N), np.float32)
    for core in range(8):
        b, s = core // 2, core % 2
        # out dram: [128, CT, NB, 512] bf16 -> [C, NQ]
        o = np.asarray(results[core]["out"]).astype(np.float32)
        out[b, :, s * NQ : (s + 1) * NQ] = (
            o.transpose(1, 0, 2, 3).reshape(C, NQ)
        )
    return out.reshape(B, C, H, W)


_cache = threading.Lock()
_program = None


def get_program():
    global _program
    with _cache:
        if _program is None:
            _program = build_program()
    return _program


def run(inputs, **kwargs):
    nc = get_program()
    in_maps = prep_inputs(inputs)
    res = run_bass_kernel_spmd(nc, in_maps, core_ids=list(range(8)), **kwargs)
    return assemble_output(res.results), res


def kernel(**inputs):
    out, _ = run(inputs)
    return out


# revision 19
# speedup vs baseline: 1.1261x; 1.1261x over previous
"""Trainium2 Bass kernel for PVT-style MHSA with spatial reduction.

Problem (hardcoded): B=4, C=384, H=W=64, NH=8 heads, HD=48, SR=2.
  q = Wq@x;  xsr = conv2x2s2(x, Wsr)+bsr;  k = (Wk@xsr + pos)*scale;  v = Wv@xsr
  attn = softmax(q^T k);  out = Wp@(v attn) + bp

Sharding: 8 cores = (batch b, query-half s).  Each core computes the full
conv/k/v for its batch (duplicated across the 2 cores of a batch) and
attention + projection for its 2048 queries.  No collectives.

v2 changes vs v1:
  - exp split across engines: most key-tiles take ACT exp; `n_dve` tiles per
    block take a Schraudolph bit-trick exp on the DVE (one fused
    tensor_scalar: int16(x*A+B) whose bits are read back as bf16).
  - reciprocal_approx_fast instead of the iterative DVE reciprocal
    (3354ns -> ~660ns per rowsum row).
  - bf16 activations on the q/conv inputs + all attention-side tensors
    (vt/e/o/proj); k-chain (wk/wv/xsr) kept fp32r for logit accuracy.
  - bf16 output, DMA'd per query block on alternating rings.

Device notes (kept from v1):
  - heads padded 48 -> 64 channels; head-pair hp occupies one 128-row tile.
  - attention computed transposed: attnT[m, n]; key-axis softmax reduction
    rides the AV matmul via an all-ones column in v^T at head-local col 32;
    QK^T pairs row-packed, AV pairs col-packed via tile_position.
  - rowsum reciprocal DRAM-bounce partition-broadcast, one tensor_mul.
"""

import threading

import numpy as np
import ml_dtypes

import concourse.bass as bass
import concourse.mybir as mybir
import concourse.tile as tile
from concourse import bacc
from concourse.bass import ts
from concourse.bass_utils import run_bass_kernel_spmd

B, C, H, W = 4, 384, 64, 64
NH, HD, SR = 8, 48, 2
SCALE = HD ** -0.5
Hs, Ws = H // SR, W // SR
NK = Hs * Ws            # 1024 keys
N = H * W               # 4096 queries / batch
NQ = N // 2             # 2048 queries / core
CT = C // 128           # 3 c-tiles
HP = NH // 2            # 4 head-pair tiles
NB = NQ // 512          # 4 query blocks / core
MT = NK // 128          # 8 key tiles

F32 = mybir.dt.float32
F32R = mybir.dt.float32r
BF16 = mybir.dt.bfloat16
I16 = mybir.dt.int16
AF = mybir.ActivationFunctionType
ALU = mybir.AluOpType

# Schraudolph exp in bf16-bit space: exp(x) ~ bits_bf16(int16(x*A + B))
EXP_A = 128.0 / float(np.log(2.0))
EXP_B = 127.0 * 128.0 - 128.0 * 0.04368

DEFAULT_CFG = dict(
    psa_bufs=2, qk_bufs=2, av_bufs=2, pr_bufs=2, e_bufs=2, r_bufs=3, dr_bufs=3,
    n_dve=3, debug=False,
)
# which key-tiles (mi) run the DVE bit-trick exp, by n_dve
DVE_MI = {0: (), 1: (3,), 2: (2, 5), 3: (1, 4, 6), 4: (1, 3, 5, 7)}


def build_program(**cfg):
    cfg = {**DEFAULT_CFG, **cfg}
    dve_mi = DVE_MI[cfg["n_dve"]]
    nc = bacc.Bacc(None, target_bir_lowering=False)

    xf = nc.dram_tensor("xf", [128, 2, CT, 4, 512], BF16, kind="ExternalInput")
    xq = nc.dram_tensor("xq", [128, NB, CT, 512], BF16, kind="ExternalInput")
    wq = nc.dram_tensor("wq", [128, CT, 512], BF16, kind="ExternalInput")
    wk = nc.dram_tensor("wk", [128, CT, 512], BF16, kind="ExternalInput")
    wv = nc.dram_tensor("wv", [128, CT, 512], BF16, kind="ExternalInput")
    wsr = nc.dram_tensor("wsr", [128, 12, C], BF16, kind="ExternalInput")
    wp = nc.dram_tensor("wp", [128, 4, C], BF16, kind="ExternalInput")
    pos = nc.dram_tensor("pos", [128, HP, NK], F32, kind="ExternalInput")
    bsr = nc.dram_tensor("bsr", [128, CT], F32, kind="ExternalInput")
    bp = nc.dram_tensor("bp", [128, CT], F32, kind="ExternalInput")
    out = nc.dram_tensor("out", [128, CT, NB, 512], BF16, kind="ExternalOutput")
    if cfg["debug"]:
        xsr_d = nc.dram_tensor("xsr_d", [128, CT, NK], BF16, kind="ExternalOutput")
        q_d = nc.dram_tensor("q_d", [128, HP, NQ], BF16, kind="ExternalOutput")
        k_d = nc.dram_tensor("k_d", [128, HP, NK], BF16, kind="ExternalOutput")
        vt_d = nc.dram_tensor("vt_d", [128, MT, 512], BF16, kind="ExternalOutput")
        e_d = nc.dram_tensor("e_d", [128, MT, 1024], BF16, kind="ExternalOutput")
        rb_d = nc.dram_tensor("rb_d", [128, 512], F32, kind="ExternalOutput")
        o_d = nc.dram_tensor("o_d", [128, HP, NQ], BF16, kind="ExternalOutput")

    with tile.TileContext(nc) as tc:
        with (
            tc.tile_pool(name="constp", bufs=1) as constp,
            tc.tile_pool(name="aload", bufs=1) as aload,
            tc.tile_pool(name="epool", bufs=cfg["e_bufs"]) as epool,
            tc.tile_pool(name="rpool", bufs=cfg["r_bufs"]) as rpool,
            tc.tile_pool(name="opool", bufs=2) as opool,
            tc.tile_pool(name="drp", bufs=cfg["dr_bufs"], space="DRAM") as drp,
            tc.tile_pool(name="psA", bufs=cfg["psa_bufs"], space="PSUM") as psA,
            tc.tile_pool(name="qkps", bufs=cfg["qk_bufs"], space="PSUM") as qkps,
            tc.tile_pool(name="avps", bufs=cfg["av_bufs"], space="PSUM") as avps,
        ):
            wk_sb = constp.tile([128, CT, 512], BF16, name="wk_sb")
            wv_sb = constp.tile([128, CT, 512], BF16, name="wv_sb")
            wp_sb = constp.tile([128, 4, C], BF16, name="wp_sb")
            bsr_sb = constp.tile([128, CT], F32, name="bsr_sb")
            bp_sb = constp.tile([128, CT], F32, name="bp_sb")
            q_sb = constp.tile([128, HP, NQ], BF16, name="q_sb")
            k_sb = constp.tile([128, HP, NK], BF16, name="k_sb")
            vt_sb = constp.tile([128, MT, 512], BF16, name="vt_sb")
            o_sb = constp.tile([128, HP, NQ], BF16, name="o_sb")

            xf_sb = aload.tile([128, 2, CT, 4, 512], BF16, name="xf_sb")
            xq_sb = aload.tile([128, NB, CT, 512], BF16, name="xq_sb")
            wq_sb = aload.tile([128, CT, 512], BF16, name="wq_sb")
            wsr_sb = aload.tile([128, 12, C], BF16, name="wsr_sb")
            pos_sb = aload.tile([128, HP, NK], F32, name="pos_sb")
            xsr_sb = aload.tile([128, CT, NK], BF16, name="xsr_sb")

            # ACT HWDGE ring: weights/bias/pos (ACT is idle until exps)
            nc.scalar.dma_start(wsr_sb[:], wsr[:])
            nc.scalar.dma_start(wk_sb[:], wk[:])
            nc.scalar.dma_start(wq_sb[:], wq[:])
            nc.scalar.dma_start(bsr_sb[:], bsr[:])
            nc.scalar.dma_start(pos_sb[:], pos[:])
            nc.scalar.dma_start(wv_sb[:], wv[:])
            nc.scalar.dma_start(wp_sb[:], wp[:])
            nc.scalar.dma_start(bp_sb[:], bp[:])
            # SP HWDGE ring: activations, ordered by first use
            nc.sync.dma_start(xf_sb[:, 0], xf[:, 0])
            nc.sync.dma_start(xf_sb[:, 1], xf[:, 1])
            nc.sync.dma_start(xq_sb[:, 0], xq[:, 0])
            for nb in range(1, NB):
                nc.sync.dma_start(xq_sb[:, nb], xq[:, nb])

            def emit_conv(mb):
                for ot in range(CT):
                    p = psA.tile([128, 512], F32, name="pa", tag="pa")
                    n_mm = 0
                    for didj in range(4):
                        for ci in range(CT):
                            nc.tensor.matmul(
                                p[:],
                                wsr_sb[:, didj * CT + ci, ts(ot, 128)],
                                xf_sb[:, mb, ci, didj, :],
                                start=(n_mm == 0),
                                stop=(n_mm == 11),
                            )
                            n_mm += 1
                    nc.vector.tensor_scalar_add(
                        xsr_sb[:, ot, ts(mb, 512)], p[:], bsr_sb[:, ot : ot + 1]
                    )

            def emit_k(hp, mb):
                p = psA.tile([128, 512], F32, name="pa", tag="pa")
                for ci in range(CT):
                    nc.tensor.matmul(
                        p[:],
                        wk_sb[:, ci, ts(hp, 128)],
                        xsr_sb[:, ci, ts(mb, 512)],
                        start=(ci == 0),
                        stop=(ci == CT - 1),
                    )
                nc.vector.tensor_add(
                    k_sb[:, hp, ts(mb, 512)], p[:], pos_sb[:, hp, ts(mb, 512)]
                )

            def emit_q(ot, nb):
                p = psA.tile([128, 512], F32, name="pa", tag="pa")
                for ci in range(CT):
                    nc.tensor.matmul(
                        p[:],
                        wq_sb[:, ci, ts(ot, 128)],
                        xq_sb[:, nb, ci, :],
                        start=(ci == 0),
                        stop=(ci == CT - 1),
                    )
                nc.any.tensor_copy(q_sb[:, ot, ts(nb, 512)], p[:])

            def emit_vt(mi):
                p = psA.tile([128, 512], F32, name="pa", tag="pa")
                for ci in range(CT):
                    nc.tensor.matmul(
                        p[:],
                        xsr_sb[:, ci, ts(mi, 128)],
                        wv_sb[:, ci, :],
                        start=(ci == 0),
                        stop=(ci == CT - 1),
                    )
                nc.any.tensor_copy(vt_sb[:, mi, :], p[:])
                base = vt_sb[:]
                ones_ap = bass.AP(
                    tensor=base.tensor,
                    offset=base.offset + mi * 512 + 32,
                    ap=[base.ap[0], [64, NH]],
                )
                nc.gpsimd.memset(ones_ap, 1.0)

            blocks = [(nb, hp) for nb in range(NB) for hp in range(HP)]
            NBLK = len(blocks)
            e_tiles = [None] * NBLK
            oav_tiles = [None] * NBLK
            rb_tiles = [None] * NBLK

            def emit_qk_exp(b, mi):
                nb, hp = blocks[b]
                e_sb = e_tiles[b]
                qk = qkps.tile([128, 1024], F32, name="qk", tag="qk")
                nc.tensor.matmul(
                    qk[:, 0:512],
                    k_sb[0:64, hp, ts(mi, 128)],
                    q_sb[0:64, hp, ts(nb, 512)],
                    start=True,
                    stop=True,
                    tile_position=(0, 0),
                )
                nc.tensor.matmul(
                    qk[:, 512:1024],
                    k_sb[64:128, hp, ts(mi, 128)],
                    q_sb[64:128, hp, ts(nb, 512)],
                    start=True,
                    stop=True,
                    tile_position=(64, 0),
                )
                if mi in dve_mi:
                    nc.vector.tensor_scalar(
                        e_sb[:, mi, :].bitcast(I16),
                        qk[:],
                        EXP_A,
                        EXP_B,
                        ALU.mult,
                        ALU.add,
                    )
                else:
                    nc.scalar.activation(
                        out=e_sb[:, mi, :], in_=qk[:], func=AF.Exp
                    )

            def emit_av(b, mi):
                nb, hp = blocks[b]
                e_sb = e_tiles[b]
                oav = oav_tiles[b]
                nc.tensor.matmul(
                    oav[0:64, :],
                    vt_sb[:, mi, 128 * hp : 128 * hp + 64],
                    e_sb[:, mi, 0:512],
                    start=(mi == 0),
                    stop=(mi == MT - 1),
                    tile_position=(0, 0),
                    skip_group_check=True,
                )
                nc.tensor.matmul(
                    oav[64:128, :],
                    vt_sb[:, mi, 128 * hp + 64 : 128 * (hp + 1)],
                    e_sb[:, mi, 512:1024],
                    start=(mi == 0),
                    stop=(mi == MT - 1),
                    tile_position=(0, 64),
                    skip_group_check=True,
                )

            def emit_rowsum(b):
                # copy the two rowsum rows out of PSUM, DRAM-bounce them into
                # a 128-partition broadcast; the reciprocal happens
                # full-partition-aligned later (custom-DVE ops silently
                # corrupt on HW with partition-offset APs).
                oav = oav_tiles[b]
                r2 = rpool.tile([33, 512], F32, name="r2", tag="r2")
                nc.vector.tensor_copy(r2[0:1, :], oav[32:33, :])
                nc.vector.tensor_copy(r2[32:33, :], oav[96:97, :])
                r2d = drp.tile([2, 512], F32, name="r2d", tag="r2d")
                nc.gpsimd.dma_start(r2d[0:1], r2[0:1, :])
                nc.gpsimd.dma_start(r2d[1:2], r2[32:33, :])
                rb = rpool.tile([128, 512], F32, name="rb", tag="rb")
                nc.gpsimd.dma_start(
                    rb[0:64, :].unsqueeze(1),
                    r2d[0:1, :].partition_broadcast(64),
                )
                nc.gpsimd.dma_start(
                    rb[64:128, :].unsqueeze(1),
                    r2d[1:2, :].partition_broadcast(64),
                )
                rb_tiles[b] = rb

            def emit_mul_proj(b):
                nb, hp = blocks[b]
                rbi = rpool.tile([128, 512], F32, name="rbi", tag="rbi")
                nc.vector.reciprocal_approx_fast(out=rbi[:], in_=rb_tiles[b][:])
                nc.vector.tensor_mul(
                    o_sb[:, hp, ts(nb, 512)], oav_tiles[b][:], rbi[:]
                )
                if hp == HP - 1:
                    outp = opool.tile(
                        [128, CT, 512], BF16, name="outp", tag="outp"
                    )
                    for ot in range(CT):
                        p = psA.tile([128, 512], F32, name="pp", tag="pa")
                        for d in range(4):
                            nc.tensor.matmul(
                                p[:],
                                wp_sb[:, d, ts(ot, 128)],
                                o_sb[:, d, ts(nb, 512)],
                                start=(d == 0),
                                stop=(d == 3),
                            )
                        nc.any.tensor_scalar_add(
                            outp[:, ot, :], p[:], bp_sb[:, ot : ot + 1]
                        )
                    ring = nc.sync if nb % 2 == 0 else nc.scalar
                    ring.dma_start(out[:, :, nb], outp[:])

            # ---- interleaved emission -----------------------------------
            # prologue: just enough of phase A to unblock block 0
            emit_conv(0)
            emit_conv(1)
            emit_k(0, 0)
            emit_k(0, 1)
            emit_q(0, 0)
            # per-step leftover phase-A work, emitted after the step's
            # attention so it fills PE gaps while staying ahead of demand
            rest_q = [(hp, nb) for nb in range(1, NB) for hp in range(HP)]
            chunks = {
                0: [lambda: [emit_vt(mi) for mi in range(4)],
                    lambda: emit_k(1, 0), lambda: emit_k(1, 1),
                    lambda: emit_q(1, 0),
                    lambda: [emit_vt(mi) for mi in range(4, MT)]],
                1: [lambda: emit_k(2, 0), lambda: emit_k(2, 1),
                    lambda: emit_q(2, 0)],
                2: [lambda: emit_k(3, 0), lambda: emit_k(3, 1),
                    lambda: emit_q(3, 0)],
            }
            for i, (hp, nb) in enumerate(rest_q):
                chunks.setdefault(3 + i, []).append(
                    lambda hp=hp, nb=nb: emit_q(hp, nb)
                )

            for s in range(NBLK + 2):
                if s < NBLK:
                    e_tiles[s] = epool.tile(
                        [128, MT, 1024], BF16, name="e", tag="e"
                    )
                if 0 <= s - 1 < NBLK:
                    oav_tiles[s - 1] = avps.tile(
                        [128, 512], F32, name="oav", tag="oav"
                    )
                for mi in range(MT):
                    if s < NBLK:
                        emit_qk_exp(s, mi)
                    if 0 <= s - 1 < NBLK:
                        emit_av(s - 1, mi)
                if 0 <= s - 1 < NBLK:
                    emit_rowsum(s - 1)
                if 0 <= s - 2 < NBLK:
                    emit_mul_proj(s - 2)
                for fn in chunks.get(s, []):
                    fn()
                if cfg["debug"] and s == 1:
                    nc.sync.dma_start(e_d[:], e_tiles[0][:])
                    nc.sync.dma_start(rb_d[:], rb_tiles[0][:])
            if cfg["debug"]:
                nc.sync.dma_start(xsr_d[:], xsr_sb[:])
                nc.sync.dma_start(q_d[:], q_sb[:])
                nc.sync.dma_start(k_d[:], k_sb[:])
                nc.sync.dma_start(vt_d[:], vt_sb[:])
                nc.sync.dma_start(o_d[:], o_sb[:])

    nc.compile()
    return nc


def _pad_cols(w):
    """[C, C] weight -> [C, 512]: col 64h+j = w[48h+j, :] (j < 48)."""
    wt = np.zeros((C, NH * 64), np.float32)
    for h in range(NH):
        wt[:, 64 * h : 64 * h + HD] = w[HD * h : HD * (h + 1), :].T
    return wt


# v/proj head-local channel placement: the ones column sits at local col 32 so
# the rowsum lands on a 32-aligned PSUM partition; channel d -> col d (d<32)
# else d+1
_VCOL = np.array([d if d < 32 else d + 1 for d in range(HD)])


def _ctile(w):
    """[C, F] -> [128, CT, F] (partition-major c-tiles)."""
    return np.ascontiguousarray(w.reshape(CT, 128, -1).transpose(1, 0, 2))


def _bf16(a):
    return np.ascontiguousarray(a).astype(ml_dtypes.bfloat16)


def prep_inputs(inputs):
    x = np.ascontiguousarray(np.asarray(inputs["x"], np.float32))
    Wq = np.asarray(inputs["Wq"], np.float32)
    Wk = np.asarray(inputs["Wk"], np.float32)
    Wv = np.asarray(inputs["Wv"], np.float32)
    Wsr = np.asarray(inputs["Wsr"], np.float32)
    bsr = np.asarray(inputs["bsr"], np.float32)
    Wp = np.asarray(inputs["Wp"], np.float32)
    bp = np.asarray(inputs["bp"], np.float32)
    rel_h = np.asarray(inputs["rel_h"], np.float32)
    rel_w = np.asarray(inputs["rel_w"], np.float32)

    wq_t = _bf16(_ctile(_pad_cols(Wq)))
    wk_t = _bf16(_ctile(_pad_cols(Wk) * SCALE))
    wv_pad = np.zeros((C, NH * 64), np.float32)
    for h in range(NH):
        wv_pad[:, 64 * h + _VCOL] = Wv[HD * h : HD * (h + 1), :].T
    wv_t = _bf16(_ctile(wv_pad))
    # conv weights: rows ordered (di, dj, c) -> [128, 12, C] (didj, ci) tiles
    wsr_t = _bf16(
        Wsr.transpose(2, 3, 1, 0).reshape(12, 128, C).transpose(1, 0, 2)
    )
    # proj weights: row 64h + vcol(j) = Wp[:, 48h+j] -> [128, 4, C]
    wp_t = np.zeros((NH * 64, C), np.float32)
    for h in range(NH):
        wp_t[64 * h + _VCOL, :] = Wp[:, HD * h : HD * (h + 1)].T
    wp_t = _bf16(wp_t.reshape(4, 128, C).transpose(1, 0, 2))
    # positional bias, pre-scaled, padded to 64-channel heads -> [128, HP, NK]
    pos_flat = (rel_h + rel_w).reshape(NH, HD, NK).astype(np.float32) * SCALE
    pos_t = np.zeros((NH * 64, NK), np.float32)
    for h in range(NH):
        pos_t[64 * h : 64 * h + HD, :] = pos_flat[h]
    pos_t = np.ascontiguousarray(pos_t.reshape(HP, 128, NK).transpose(1, 0, 2))
    bsr_t = np.ascontiguousarray(bsr.reshape(CT, 128).T)
    bp_t = np.ascontiguousarray(bp.reshape(CT, 128).T)

    in_maps = []
    for core in range(8):
        b, s = core // 2, core % 2
        xb = x[b].reshape(C, N)
        # conv planes, deinterleaved: [p, mb, ci, didj, ph_local*32+pw] =
        # x[ci*128+p, 2*(mb*16+ph_local)+di, 2*pw+dj]
        xf_t = _bf16(
            x[b]
            .reshape(C, Hs, SR, Ws, SR)          # c, ph, di, pw, dj
            .transpose(0, 2, 4, 1, 3)            # c, di, dj, ph, pw
            .reshape(C, 4, 2, Hs // 2, Ws)       # c, didj, mb, ph_l, pw
            .transpose(0, 2, 1, 3, 4)            # c, mb, didj, ph_l, pw
            .reshape(CT, 128, 2, 4, 512)
            .transpose(1, 2, 0, 3, 4)
        )
        xq_t = _bf16(
            xb[:, s * NQ : (s + 1) * NQ]
            .reshape(CT, 128, NB, 512)
            .transpose(1, 2, 0, 3)
        )
        in_maps.append(
            {
                "xf": xf_t,
                "xq": xq_t,
                "wq": wq_t,
                "wk": wk_t,
                "wv": wv_t,
                "wsr": wsr_t,
                "wp": wp_t,
                "pos": pos_t,
                "bsr": bsr_t,
                "bp": bp_t,
            }
        )
    return in_maps


def assemble_output(results):
    out = np.empty((B, C, N), np.float32)
    for core in range(8):
        b, s = core // 2, core % 2
        # out dram: [128, CT, NB, 512] bf16 -> [C, NQ]
        o = np.asarray(results[core]["out"]).astype(np.float32)
        out[b, :, s * NQ : (s + 1) * NQ] = (
            o.transpose(1, 0, 2, 3).reshape(C, NQ)
        )
    return out.reshape(B, C, H, W)


_cache = threading.Lock()
_program = None


def get_program():
    global _program
    with _cache:
        if _program is None:
            _program = build_program()
    return _program


def run(inputs, **kwargs):
    nc = get_program()
    in_maps = prep_inputs(inputs)
    res = run_bass_kernel_spmd(nc, in_maps, core_ids=list(range(8)), **kwargs)
    return assemble_output(res.results), res


def kernel(**inputs):
    out, _ = run(inputs)
    return out


# revision 20
# speedup vs baseline: 1.2409x; 1.1019x over previous
"""Trainium2 Bass kernel for PVT-style MHSA with spatial reduction.

Problem (hardcoded): B=4, C=384, H=W=64, NH=8 heads, HD=48, SR=2.
  q = Wq@x;  xsr = conv2x2s2(x, Wsr)+bsr;  k = (Wk@xsr + pos)*scale;  v = Wv@xsr
  attn = softmax(q^T k);  out = Wp@(v attn) + bp

Sharding: 8 cores = (batch b, query-half s).  Each core computes the full
conv/k/v for its batch (duplicated across the 2 cores of a batch) and
attention + projection for its 2048 queries.  No collectives.

v2 changes vs v1:
  - exp split across engines: most key-tiles take ACT exp; `n_dve` tiles per
    block take a Schraudolph bit-trick exp on the DVE (one fused
    tensor_scalar: int16(x*A+B) whose bits are read back as bf16).
  - reciprocal_approx_fast instead of the iterative DVE reciprocal
    (3354ns -> ~660ns per rowsum row).
  - bf16 activations on the q/conv inputs + all attention-side tensors
    (vt/e/o/proj); k-chain (wk/wv/xsr) kept fp32r for logit accuracy.
  - bf16 output, DMA'd per query block on alternating rings.

Device notes (kept from v1):
  - heads padded 48 -> 64 channels; head-pair hp occupies one 128-row tile.
  - attention computed transposed: attnT[m, n]; key-axis softmax reduction
    rides the AV matmul via an all-ones column in v^T at head-local col 32;
    QK^T pairs row-packed, AV pairs col-packed via tile_position.
  - rowsum reciprocal DRAM-bounce partition-broadcast, one tensor_mul.
"""

import threading

import numpy as np
import ml_dtypes

import concourse.bass as bass
import concourse.mybir as mybir
import concourse.tile as tile
from concourse import bacc
from concourse.bass import ts
from concourse.bass_utils import run_bass_kernel_spmd

B, C, H, W = 4, 384, 64, 64
NH, HD, SR = 8, 48, 2
SCALE = HD ** -0.5
Hs, Ws = H // SR, W // SR
NK = Hs * Ws            # 1024 keys
N = H * W               # 4096 queries / batch
NQ = N // 2             # 2048 queries / core
CT = C // 128           # 3 c-tiles
HP = NH // 2            # 4 head-pair tiles
NB = NQ // 512          # 4 query blocks / core
MT = NK // 128          # 8 key tiles

F32 = mybir.dt.float32
F32R = mybir.dt.float32r
BF16 = mybir.dt.bfloat16
I16 = mybir.dt.int16
AF = mybir.ActivationFunctionType
ALU = mybir.AluOpType

# Schraudolph exp in bf16-bit space: exp(x) ~ bits_bf16(int16(x*A + B))
EXP_A = 128.0 / float(np.log(2.0))
EXP_B = 127.0 * 128.0 - 128.0 * 0.04368

DEFAULT_CFG = dict(
    psa_bufs=2, qk_bufs=2, av_bufs=2, pr_bufs=2, e_bufs=2, r_bufs=3, dr_bufs=3,
    n_dve=3, debug=False,
)
# which key-tiles (mi) run the DVE bit-trick exp, by n_dve
DVE_MI = {0: (), 1: (3,), 2: (2, 5), 3: (1, 4, 6), 4: (1, 3, 5, 7)}


def build_program(**cfg):
    cfg = {**DEFAULT_CFG, **cfg}
    dve_mi = DVE_MI[cfg["n_dve"]]
    nc = bacc.Bacc(None, target_bir_lowering=False)

    xf = nc.dram_tensor("xf", [128, 2, CT, 4, 512], BF16, kind="ExternalInput")
    xq = nc.dram_tensor("xq", [128, NB, CT, 512], BF16, kind="ExternalInput")
    wq = nc.dram_tensor("wq", [128, CT, 512], BF16, kind="ExternalInput")
    wk = nc.dram_tensor("wk", [128, CT, 512], BF16, kind="ExternalInput")
    wv = nc.dram_tensor("wv", [128, CT, 512], BF16, kind="ExternalInput")
    wsr = nc.dram_tensor("wsr", [128, 12, C], BF16, kind="ExternalInput")
    wp = nc.dram_tensor("wp", [128, 4, C], BF16, kind="ExternalInput")
    pos = nc.dram_tensor("pos", [128, HP, NK], F32, kind="ExternalInput")
    bsr = nc.dram_tensor("bsr", [128, CT], F32, kind="ExternalInput")
    bp = nc.dram_tensor("bp", [128, CT], F32, kind="ExternalInput")
    out = nc.dram_tensor("out", [128, CT, NB, 512], BF16, kind="ExternalOutput")
    if cfg["debug"]:
        xsr_d = nc.dram_tensor("xsr_d", [128, CT, NK], BF16, kind="ExternalOutput")
        q_d = nc.dram_tensor("q_d", [128, HP, NQ], BF16, kind="ExternalOutput")
        k_d = nc.dram_tensor("k_d", [128, HP, NK], BF16, kind="ExternalOutput")
        vt_d = nc.dram_tensor("vt_d", [128, MT, 512], BF16, kind="ExternalOutput")
        e_d = nc.dram_tensor("e_d", [128, MT, 1024], BF16, kind="ExternalOutput")
        rb_d = nc.dram_tensor("rb_d", [128, 512], F32, kind="ExternalOutput")
        o_d = nc.dram_tensor("o_d", [128, HP, NQ], BF16, kind="ExternalOutput")

    with tile.TileContext(nc) as tc:
        with (
            tc.tile_pool(name="constp", bufs=1) as constp,
            tc.tile_pool(name="aload", bufs=1) as aload,
            tc.tile_pool(name="epool", bufs=cfg["e_bufs"]) as epool,
            tc.tile_pool(name="rpool", bufs=cfg["r_bufs"]) as rpool,
            tc.tile_pool(name="opool", bufs=2) as opool,
            tc.tile_pool(name="drp", bufs=cfg["dr_bufs"], space="DRAM") as drp,
            tc.tile_pool(name="psA", bufs=cfg["psa_bufs"], space="PSUM") as psA,
            tc.tile_pool(name="qkps", bufs=cfg["qk_bufs"], space="PSUM") as qkps,
            tc.tile_pool(name="avps", bufs=cfg["av_bufs"], space="PSUM") as avps,
        ):
            wk_sb = constp.tile([128, CT, 512], BF16, name="wk_sb")
            wv_sb = constp.tile([128, CT, 512], BF16, name="wv_sb")
            wp_sb = constp.tile([128, 4, C], BF16, name="wp_sb")
            bsr_sb = constp.tile([128, CT], F32, name="bsr_sb")
            bp_sb = constp.tile([128, CT], F32, name="bp_sb")
            q_sb = constp.tile([128, HP, NQ], BF16, name="q_sb")
            k_sb = constp.tile([128, HP, NK], BF16, name="k_sb")
            vt_sb = constp.tile([128, MT, 512], BF16, name="vt_sb")
            o_sb = constp.tile([128, HP, NQ], BF16, name="o_sb")

            xf_sb = aload.tile([128, 2, CT, 4, 512], BF16, name="xf_sb")
            xq_sb = aload.tile([128, NB, CT, 512], BF16, name="xq_sb")
            wq_sb = aload.tile([128, CT, 512], BF16, name="wq_sb")
            wsr_sb = aload.tile([128, 12, C], BF16, name="wsr_sb")
            pos_sb = aload.tile([128, HP, NK], F32, name="pos_sb")
            xsr_sb = aload.tile([128, CT, NK], BF16, name="xsr_sb")

            # ACT HWDGE ring: weights/bias/pos (ACT is idle until exps)
            nc.scalar.dma_start(wsr_sb[:], wsr[:])
            nc.scalar.dma_start(wk_sb[:], wk[:])
            nc.scalar.dma_start(wq_sb[:], wq[:])
            nc.scalar.dma_start(bsr_sb[:], bsr[:])
            nc.scalar.dma_start(pos_sb[:], pos[:])
            nc.scalar.dma_start(wv_sb[:], wv[:])
            nc.scalar.dma_start(wp_sb[:], wp[:])
            nc.scalar.dma_start(bp_sb[:], bp[:])
            # SP HWDGE ring: activations, ordered by first use
            for ci in range(CT):
                nc.sync.dma_start(xf_sb[:, 0, ci], xf[:, 0, ci])
            for ci in range(CT):
                nc.sync.dma_start(xf_sb[:, 1, ci], xf[:, 1, ci])
            nc.sync.dma_start(xq_sb[:, 0], xq[:, 0])
            for nb in range(1, NB):
                nc.sync.dma_start(xq_sb[:, nb], xq[:, nb])

            def emit_conv(mb):
                for ot in range(CT):
                    p = psA.tile([128, 512], F32, name="pa", tag="pa")
                    n_mm = 0
                    for ci in range(CT):
                        for didj in range(4):
                            nc.tensor.matmul(
                                p[:],
                                wsr_sb[:, didj * CT + ci, ts(ot, 128)],
                                xf_sb[:, mb, ci, didj, :],
                                start=(n_mm == 0),
                                stop=(n_mm == 11),
                            )
                            n_mm += 1
                    nc.vector.tensor_scalar_add(
                        xsr_sb[:, ot, ts(mb, 512)], p[:], bsr_sb[:, ot : ot + 1]
                    )

            def emit_k(hp, mb):
                p = psA.tile([128, 512], F32, name="pa", tag="pa")
                for ci in range(CT):
                    nc.tensor.matmul(
                        p[:],
                        wk_sb[:, ci, ts(hp, 128)],
                        xsr_sb[:, ci, ts(mb, 512)],
                        start=(ci == 0),
                        stop=(ci == CT - 1),
                    )
                nc.vector.tensor_add(
                    k_sb[:, hp, ts(mb, 512)], p[:], pos_sb[:, hp, ts(mb, 512)]
                )

            def emit_q(ot, nb):
                p = psA.tile([128, 512], F32, name="pa", tag="pa")
                for ci in range(CT):
                    nc.tensor.matmul(
                        p[:],
                        wq_sb[:, ci, ts(ot, 128)],
                        xq_sb[:, nb, ci, :],
                        start=(ci == 0),
                        stop=(ci == CT - 1),
                    )
                nc.scalar.copy(q_sb[:, ot, ts(nb, 512)], p[:])

            def emit_vt(mi):
                p = psA.tile([128, 512], F32, name="pa", tag="pa")
                for ci in range(CT):
                    nc.tensor.matmul(
                        p[:],
                        xsr_sb[:, ci, ts(mi, 128)],
                        wv_sb[:, ci, :],
                        start=(ci == 0),
                        stop=(ci == CT - 1),
                    )
                nc.any.tensor_copy(vt_sb[:, mi, :], p[:])
                base = vt_sb[:]
                ones_ap = bass.AP(
                    tensor=base.tensor,
                    offset=base.offset + mi * 512 + 32,
                    ap=[base.ap[0], [64, NH]],
                )
                nc.gpsimd.memset(ones_ap, 1.0)

            blocks = [(nb, hp) for nb in range(NB) for hp in range(HP)]
            NBLK = len(blocks)
            e_tiles = [None] * NBLK
            oav_tiles = [None] * NBLK
            rb_tiles = [None] * NBLK

            def emit_qk_exp(b, mi):
                nb, hp = blocks[b]
                e_sb = e_tiles[b]
                qk = qkps.tile([128, 1024], F32, name="qk", tag="qk")
                nc.tensor.matmul(
                    qk[:, 0:512],
                    k_sb[0:64, hp, ts(mi, 128)],
                    q_sb[0:64, hp, ts(nb, 512)],
                    start=True,
                    stop=True,
                    tile_position=(0, 0),
                )
                nc.tensor.matmul(
                    qk[:, 512:1024],
                    k_sb[64:128, hp, ts(mi, 128)],
                    q_sb[64:128, hp, ts(nb, 512)],
                    start=True,
                    stop=True,
                    tile_position=(64, 0),
                )
                if mi in dve_mi:
                    nc.vector.tensor_scalar(
                        e_sb[:, mi, :].bitcast(I16),
                        qk[:],
                        EXP_A,
                        EXP_B,
                        ALU.mult,
                        ALU.add,
                    )
                else:
                    nc.scalar.activation(
                        out=e_sb[:, mi, :], in_=qk[:], func=AF.Exp
                    )

            def emit_av(b, mi):
                nb, hp = blocks[b]
                e_sb = e_tiles[b]
                oav = oav_tiles[b]
                nc.tensor.matmul(
                    oav[0:64, :],
                    vt_sb[:, mi, 128 * hp : 128 * hp + 64],
                    e_sb[:, mi, 0:512],
                    start=(mi == 0),
                    stop=(mi == MT - 1),
                    tile_position=(0, 0),
                    skip_group_check=True,
                )
                nc.tensor.matmul(
                    oav[64:128, :],
                    vt_sb[:, mi, 128 * hp + 64 : 128 * (hp + 1)],
                    e_sb[:, mi, 512:1024],
                    start=(mi == 0),
                    stop=(mi == MT - 1),
                    tile_position=(0, 64),
                    skip_group_check=True,
                )

            def emit_rowsum(b):
                # elementwise 1/oav on the whole tile (full-partition-aligned:
                # custom-DVE ops silently corrupt on HW with partition-offset
                # APs); rows 32/96 hold 1/rowsum, rest is discarded. Those two
                # rows DRAM-bounce into a 128-partition broadcast.
                oav = oav_tiles[b]
                rinv = rpool.tile([128, 512], F32, name="rinv", tag="rinv")
                nc.vector.reciprocal_approx_fast(out=rinv[:], in_=oav[:])
                r2d = drp.tile([2, 512], F32, name="r2d", tag="r2d")
                nc.gpsimd.dma_start(r2d[0:1], rinv[32:33, :])
                nc.gpsimd.dma_start(r2d[1:2], rinv[96:97, :])
                rb = rpool.tile([128, 512], F32, name="rb", tag="rb")
                nc.gpsimd.dma_start(
                    rb[0:64, :].unsqueeze(1),
                    r2d[0:1, :].partition_broadcast(64),
                )
                nc.gpsimd.dma_start(
                    rb[64:128, :].unsqueeze(1),
                    r2d[1:2, :].partition_broadcast(64),
                )
                rb_tiles[b] = rb

            def emit_mul_proj(b):
                nb, hp = blocks[b]
                nc.vector.tensor_mul(
                    o_sb[:, hp, ts(nb, 512)], oav_tiles[b][:], rb_tiles[b][:]
                )
                if hp == HP - 1:
                    outp = opool.tile(
                        [128, CT, 512], BF16, name="outp", tag="outp"
                    )
                    for ot in range(CT):
                        p = psA.tile([128, 512], F32, name="pp", tag="pa")
                        for d in range(4):
                            nc.tensor.matmul(
                                p[:],
                                wp_sb[:, d, ts(ot, 128)],
                                o_sb[:, d, ts(nb, 512)],
                                start=(d == 0),
                                stop=(d == 3),
                            )
                        nc.any.tensor_scalar_add(
                            outp[:, ot, :], p[:], bp_sb[:, ot : ot + 1]
                        )
                    ring = nc.sync if nb % 2 == 0 else nc.scalar
                    ring.dma_start(out[:, :, nb], outp[:])

            # ---- interleaved emission -----------------------------------
            # prologue: just enough of phase A to unblock block 0
            emit_conv(0)
            emit_conv(1)
            emit_k(0, 0)
            emit_k(0, 1)
            emit_q(0, 0)
            # per-step leftover phase-A work, emitted after the step's
            # attention so it fills PE gaps while staying ahead of demand
            rest_q = [(hp, nb) for nb in range(1, NB) for hp in range(HP)]
            chunks = {
                0: [lambda: [emit_vt(mi) for mi in range(4)],
                    lambda: emit_k(1, 0), lambda: emit_k(1, 1),
                    lambda: emit_q(1, 0),
                    lambda: [emit_vt(mi) for mi in range(4, MT)]],
                1: [lambda: emit_k(2, 0), lambda: emit_k(2, 1),
                    lambda: emit_q(2, 0)],
                2: [lambda: emit_k(3, 0), lambda: emit_k(3, 1),
                    lambda: emit_q(3, 0)],
            }
            for i, (hp, nb) in enumerate(rest_q):
                chunks.setdefault(max(1, 1 + i), []).append(
                    lambda hp=hp, nb=nb: emit_q(hp, nb)
                )

            for s in range(NBLK + 2):
                if s < NBLK:
                    e_tiles[s] = epool.tile(
                        [128, MT, 1024], BF16, name="e", tag="e"
                    )
                if 0 <= s - 1 < NBLK:
                    oav_tiles[s - 1] = avps.tile(
                        [128, 512], F32, name="oav", tag="oav"
                    )
                for mi in range(MT):
                    if s < NBLK:
                        emit_qk_exp(s, mi)
                    if 0 <= s - 1 < NBLK:
                        emit_av(s - 1, mi)
                if 0 <= s - 1 < NBLK:
                    emit_rowsum(s - 1)
                if 0 <= s - 2 < NBLK:
                    emit_mul_proj(s - 2)
                for fn in chunks.get(s, []):
                    fn()
                if cfg["debug"] and s == 1:
                    nc.sync.dma_start(e_d[:], e_tiles[0][:])
                    nc.sync.dma_start(rb_d[:], rb_tiles[0][:])
            if cfg["debug"]:
                nc.sync.dma_start(xsr_d[:], xsr_sb[:])
                nc.sync.dma_start(q_d[:], q_sb[:])
                nc.sync.dma_start(k_d[:], k_sb[:])
                nc.sync.dma_start(vt_d[:], vt_sb[:])
                nc.sync.dma_start(o_d[:], o_sb[:])

    nc.compile()
    return nc


def _pad_cols(w):
    """[C, C] weight -> [C, 512]: col 64h+j = w[48h+j, :] (j < 48)."""
    wt = np.zeros((C, NH * 64), np.float32)
    for h in range(NH):
        wt[:, 64 * h : 64 * h + HD] = w[HD * h : HD * (h + 1), :].T
    return wt


# v/proj head-local channel placement: the ones column sits at local col 32 so
# the rowsum lands on a 32-aligned PSUM partition; channel d -> col d (d<32)
# else d+1
_VCOL = np.array([d if d < 32 else d + 1 for d in range(HD)])


def _ctile(w):
    """[C, F] -> [128, CT, F] (partition-major c-tiles)."""
    return np.ascontiguousarray(w.reshape(CT, 128, -1).transpose(1, 0, 2))


def _bf16(a):
    return np.ascontiguousarray(a).astype(ml_dtypes.bfloat16)


def prep_inputs(inputs):
    x = np.ascontiguousarray(np.asarray(inputs["x"], np.float32))
    Wq = np.asarray(inputs["Wq"], np.float32)
    Wk = np.asarray(inputs["Wk"], np.float32)
    Wv = np.asarray(inputs["Wv"], np.float32)
    Wsr = np.asarray(inputs["Wsr"], np.float32)
    bsr = np.asarray(inputs["bsr"], np.float32)
    Wp = np.asarray(inputs["Wp"], np.float32)
    bp = np.asarray(inputs["bp"], np.float32)
    rel_h = np.asarray(inputs["rel_h"], np.float32)
    rel_w = np.asarray(inputs["rel_w"], np.float32)

    wq_t = _bf16(_ctile(_pad_cols(Wq)))
    wk_t = _bf16(_ctile(_pad_cols(Wk) * SCALE))
    wv_pad = np.zeros((C, NH * 64), np.float32)
    for h in range(NH):
        wv_pad[:, 64 * h + _VCOL] = Wv[HD * h : HD * (h + 1), :].T
    wv_t = _bf16(_ctile(wv_pad))
    # conv weights: rows ordered (di, dj, c) -> [128, 12, C] (didj, ci) tiles
    wsr_t = _bf16(
        Wsr.transpose(2, 3, 1, 0).reshape(12, 128, C).transpose(1, 0, 2)
    )
    # proj weights: row 64h + vcol(j) = Wp[:, 48h+j] -> [128, 4, C]
    wp_t = np.zeros((NH * 64, C), np.float32)
    for h in range(NH):
        wp_t[64 * h + _VCOL, :] = Wp[:, HD * h : HD * (h + 1)].T
    wp_t = _bf16(wp_t.reshape(4, 128, C).transpose(1, 0, 2))
    # positional bias, pre-scaled, padded to 64-channel heads -> [128, HP, NK]
    pos_flat = (rel_h + rel_w).reshape(NH, HD, NK).astype(np.float32) * SCALE
    pos_t = np.zeros((NH * 64, NK), np.float32)
    for h in range(NH):
        pos_t[64 * h : 64 * h + HD, :] = pos_flat[h]
    pos_t = np.ascontiguousarray(pos_t.reshape(HP, 128, NK).transpose(1, 0, 2))
    bsr_t = np.ascontiguousarray(bsr.reshape(CT, 128).T)
    bp_t = np.ascontiguousarray(bp.reshape(CT, 128).T)

    in_maps = []
    for core in range(8):
        b, s = core // 2, core % 2
        xb = x[b].reshape(C, N)
        # conv planes, deinterleaved: [p, mb, ci, didj, ph_local*32+pw] =
        # x[ci*128+p, 2*(mb*16+ph_local)+di, 2*pw+dj]
        xf_t = _bf16(
            x[b]
            .reshape(C, Hs, SR, Ws, SR)          # c, ph, di, pw, dj
            .transpose(0, 2, 4, 1, 3)            # c, di, dj, ph, pw
            .reshape(C, 4, 2, Hs // 2, Ws)       # c, didj, mb, ph_l, pw
            .transpose(0, 2, 1, 3, 4)            # c, mb, didj, ph_l, pw
            .reshape(CT, 128, 2, 4, 512)
            .transpose(1, 2, 0, 3, 4)
        )
        xq_t = _bf16(
            xb[:, s * NQ : (s + 1) * NQ]
            .reshape(CT, 128, NB, 512)
            .transpose(1, 2, 0, 3)
        )
        in_maps.append(
            {
                "xf": xf_t,
                "xq": xq_t,
                "wq": wq_t,
                "wk": wk_t,
                "wv": wv_t,
                "wsr": wsr_t,
                "wp": wp_t,
                "pos": pos_t,
                "bsr": bsr_t,
                "bp": bp_t,
            }
        )
    return in_maps


def assemble_output(results):
    out = np.empty((B, C, N), np.float32)
    for core in range(8):
        b, s = core // 2, core % 2
        # out dram: [128, CT, NB, 512] bf16 -> [C, NQ]
        o = np.asarray(results[core]["out"]).astype(np.float32)
        out[b, :, s * NQ : (s + 1) * NQ] = (
            o.transpose(1, 0, 2, 3).reshape(C, NQ)
        )
    return out.reshape(B, C, H, W)


_cache = threading.Lock()
_program = None


def get_program():
    global _program
    with _cache:
        if _program is None:
            _program = build_program()
    return _program


def run(inputs, **kwargs):
    nc = get_program()
    in_maps = prep_inputs(inputs)
    res = run_bass_kernel_spmd(nc, in_maps, core_ids=list(range(8)), **kwargs)
    return assemble_output(res.results), res


def kernel(**inputs):
    out, _ = run(inputs)
    return out
